# revision 32
# baseline (speedup 1.0000x reference)
"""Kimi-style MoE (8 routed experts top-2 + shared expert) on 8 Trainium2 cores.

Sharding: expert-parallel with host-side token dispatch. The gate (a tiny
[T,8] matmul + sigmoid + top-2) runs on the host during input prep. Routed
work is packed into a small number of fixed-width "slots" per core; slot
widths are chosen at prep time by a DP search over the actual expert token
counts so the total static width (CR) is minimal. A slot holds a contiguous
segment of one expert's token list (an expert may span several slots on
several cores), so the SPMD program stays static while padding is small.
Each slot ships its own expert weights/biases; slot outputs return per token
slot (bias applied, combine weight NOT applied).

The shared expert (I_SH = 22 pair tiles of 128) is split as 2.75 tiles per
core: each core owns 2 full pair tiles over all T tokens plus 3 pair tiles
over a 512-token quarter. To keep the program identical across cores, each
core receives xT with its tokens rolled so its "deep" quarter is always
columns [0:512); the host un-rolls out_s when combining. Core c (c<4) owns
quarter c of tiles 16..18; core c (>=4) owns quarter c-4 of tiles 19..21.

All matmuls run in bf16 with fp32 PSUM accumulation; outputs transfer as
fp16 to halve output DMA.
"""

import sys

for _p in ("/opt/trn_rl_repo", "/opt/pypackages"):
    if _p not in sys.path:
        sys.path.insert(0, _p)

import numpy as np
import ml_dtypes

import concourse.bass as bass
import concourse.mybir as mybir
import concourse.tile as tile
from concourse import bacc
from concourse.bass import ts
from concourse.bass_utils import run_bass_kernel_spmd

BF16 = mybir.dt.bfloat16
F16 = mybir.dt.float16
F32 = mybir.dt.float32
NP_BF16 = ml_dtypes.bfloat16

# Problem shapes (hardcoded per the contract).
B, S, D = 2, 1024, 1024
E, TOPK = 8, 2
I = 1408
N_SHARED = 2
I_SH = N_SHARED * I          # 2816
SCALE = 2.5
T = B * S                    # 2048
P = 128
NT = T // 512                # 4 free-dim tiles of 512 tokens (shared expert)
KO = D // P                  # 8 contraction subtiles over D
JR = I // P                  # 11 routed (v,g) pair tiles
SH_TILES = I_SH // P         # 22 shared pair tiles globally
SH_FULL = 2                  # full-T pair tiles per core (tiles 0..15)
SH_Q = 3                     # quarter pair tiles per core (tiles 16..21)
SH_PER_CORE = SH_FULL + SH_Q # 5 weight tiles resident per core
QW = 512                     # deep-quarter token width
DT = D // P                  # 8 output partition tiles
N_CORES = 8

ROLLS = [QW * (c % 4) for c in range(N_CORES)]

SLOT_DEFAULT = (188, 350)    # optimal for the reference input's expert counts


def _free_tiles(F):
    """Split a free-dim extent into <=512-wide tiles."""
    if F <= 512:
        return [F]
    n = (F + 511) // 512
    assert F % n == 0
    return [F // n] * n


def _body(tc, io, slots, uid=0, mq=False, xtq="gpsimd"):
    nc = tc.nc
    nslots = len(slots)
    CR = sum(slots)
    # DMA queue per routed slot: with mq, spread weight streams across the
    # scalar/vector engines' queues so the sync FIFO isn't the only lane.
    if mq:
        w1q = [nc.sync, nc.sync, nc.gpsimd][:nslots]
        wdq = [nc.gpsimd, nc.sync, nc.sync][:nslots]
    else:
        w1q = [nc.sync] * nslots
        wdq = [nc.sync] * nslots
    add = mybir.AluOpType.add
    mult = mybir.AluOpType.mult
    Silu = mybir.ActivationFunctionType.Silu
    Ident = mybir.ActivationFunctionType.Identity
    # routed free-dim tiles: (slot, offset, width)
    rsegs = []
    off = 0
    for s in range(nslots):
        for f in _free_tiles(slots[s]):
            rsegs.append((s, off, f))
            off += f

    # HAM pre-warm: the PE powers up throttled (K=4/8, ~half clock) and only
    # un-throttles after a few microseconds of sustained matmul activity.
    # Dummy matmuls on a zeroed tile during the initial DMA-fill window (PE
    # otherwise idle) move that ramp off the critical path for one-shot runs.
    if uid == 0:
        with (
            tc.tile_pool(name="warmsb", bufs=1) as wsb,
            tc.tile_pool(name="warmps", bufs=2, space="PSUM") as wps,
        ):
            wz = wsb.tile([P, 5 * P], BF16, tag="wz")
            nc.gpsimd.memset(wz[:], 0.0)
            for _ in range(6):
                pw = wps.tile([P, 512], F32, tag="pw")
                nc.tensor.matmul(pw[:], wz[:, :P], wz[:, P:], start=True, stop=True)

    with (
        tc.tile_pool(name="const", bufs=1) as cpool,
        tc.tile_pool(name="w1s", bufs=6) as w1pool,
        tc.tile_pool(name="sv", bufs=6) as svpool,
        tc.tile_pool(name="outs", bufs=6) as opool,
    ):
        # ---- resident SBUF tensors ----
        xg = cpool.tile([P, KO, CR], BF16, tag="xg")
        xT = cpool.tile([P, KO, T], BF16, tag="xT")
        b1 = [
            cpool.tile([P, 2 * JR], F32, name=f"b1_{s}", tag=f"b1_{s}")
            for s in range(nslots)
        ]
        bs1 = cpool.tile([P, 2 * SH_PER_CORE], F32, tag="bs1")
        b2 = [
            cpool.tile([P, DT], F32, name=f"b2_{s}", tag=f"b2_{s}")
            for s in range(nslots)
        ]
        bs2 = cpool.tile([P, DT], F32, tag="bs2")
        h_r = cpool.tile([P, JR, CR], BF16, tag="h_r")
        h_sf = cpool.tile([P, SH_FULL, T], BF16, tag="h_sf")
        h_sq = cpool.tile([P, SH_Q, QW], BF16, tag="h_sq")
        wdr = [
            cpool.tile([P, DT, JR, P], BF16, name=f"wdr{s}", tag=f"wdr{s}")
            for s in range(nslots)
        ]
        wds = cpool.tile([P, DT, SH_PER_CORE, P], BF16, tag="wds")

        # Queue order matters: every input DMA shares the sync-engine FIFO, so
        # issue in consumption order (xg -> w1 stream [in the loop below] ->
        # xT -> wd). Output DMAs ride the gpsimd queue to stay out of the way.
        # The narrow slot's tokens + its j=0 weights go first so the first
        # matmul group's dependencies arrive after ~0.9MB instead of ~1.6MB.
        for k in range(KO):
            nc.sync.dma_start(xg[:, k, : slots[0]], io["xg"][:, k, : slots[0]])
        w1_first = w1pool.tile([P, KO, 2 * P], BF16, name="w1_first", tag="w1")
        nc.sync.dma_start(w1_first[:], io["w1t0"][:, 0])
        for k in range(KO):
            nc.sync.dma_start(xg[:, k, slots[0] :], io["xg"][:, k, slots[0] :])
        for s in range(nslots):
            nc.sync.dma_start(b1[s][:], io[f"b1t{s}"][:])
        nc.sync.dma_start(bs1[:], io["bs1t"][:])
        for s in range(nslots):
            nc.sync.dma_start(b2[s][:], io[f"b2c{s}"][:])
        nc.sync.dma_start(bs2[:], io["bs2c"][:])

        # ---- up projections + swiglu ----
        # routed: h_r[:, j, :] over the CR gathered token slots
        with tc.tile_pool(name="upsum", bufs=4, space="PSUM") as upsum:
            for j in range(JR):
                w1t = []
                for s in range(nslots):
                    if j == 0 and s == 0:
                        w1t.append(w1_first)
                        continue
                    w = w1pool.tile([P, KO, 2 * P], BF16, name=f"w1_{s}", tag="w1")
                    w1q[s].dma_start(w[:], io[f"w1t{s}"][:, j])
                    w1t.append(w)
                for slot, off, fr in rsegs:
                    w1tile = w1t[slot]
                    b1t = b1[slot]
                    pv = upsum.tile([P, fr], F32, tag="pv")
                    pgu = upsum.tile([P, fr], F32, tag="pgu")
                    for k in range(KO):
                        nc.tensor.matmul(
                            pv[:], w1tile[:, k, :P], xg[:, k, off : off + fr],
                            start=(k == 0), stop=(k == KO - 1),
                        )
                    for k in range(KO):
                        nc.tensor.matmul(
                            pgu[:], w1tile[:, k, P:], xg[:, k, off : off + fr],
                            start=(k == 0), stop=(k == KO - 1),
                        )
                    sv = svpool.tile([P, fr], F32, tag="sv")
                    nc.scalar.activation(
                        sv[:], pv[:], Silu, bias=b1t[:, 2 * j : 2 * j + 1]
                    )
                    nc.vector.scalar_tensor_tensor(
                        h_r[:, j, off : off + fr], pgu[:],
                        b1t[:, 2 * j + 1 : 2 * j + 2], sv[:], add, mult,
                    )
            # xT is first needed here; wd only in the down phase below. The
            # sync FIFO is over-budget during routed-up (xg + w1 stream + xT
            # > compute window), so xT defaults to the gpsimd queue, which
            # is idle until the down phase and has no ring-reuse hazard for
            # this single-buffer resident tile.
            xt_eng = getattr(nc, xtq)
            for k in range(KO):
                xt_eng.dma_start(xT[:, k], io["xT"][:, k])
            # shared: 2 full tiles over all T, then 3 deep-quarter tiles
            for jj in range(SH_PER_CORE):
                w1tile = w1pool.tile([P, KO, 2 * P], BF16, tag="w1")
                nc.sync.dma_start(w1tile[:], io["ws1t"][:, jj])
                nchunk = NT if jj < SH_FULL else 1
                for t in range(nchunk):
                    pv = upsum.tile([P, 512], F32, tag="pv")
                    pgu = upsum.tile([P, 512], F32, tag="pgu")
                    for k in range(KO):
                        nc.tensor.matmul(
                            pv[:], w1tile[:, k, :P], xT[:, k, ts(t, 512)],
                            start=(k == 0), stop=(k == KO - 1),
                        )
                    for k in range(KO):
                        nc.tensor.matmul(
                            pgu[:], w1tile[:, k, P:], xT[:, k, ts(t, 512)],
                            start=(k == 0), stop=(k == KO - 1),
                        )
                    sv = svpool.tile([P, 512], F32, tag="sv")
                    nc.scalar.activation(
                        sv[:], pv[:], Silu, bias=bs1[:, 2 * jj : 2 * jj + 1]
                    )
                    if jj < SH_FULL:
                        hdst = h_sf[:, jj, ts(t, 512)]
                    else:
                        hdst = h_sq[:, jj - SH_FULL, :]
                    nc.vector.scalar_tensor_tensor(
                        hdst, pgu[:],
                        bs1[:, 2 * jj + 1 : 2 * jj + 2], sv[:], add, mult,
                    )

        # ---- down-projection weights (resident; issued after all up-phase
        # loads so they sit behind them in the sync DMA FIFO) ----
        for dt in range(DT):
            for s in range(nslots):
                wdq[s].dma_start(wdr[s][:, dt], io[f"wdr{s}"][:, dt])
            nc.sync.dma_start(wds[:, dt], io["wds"][:, dt])

        # ---- down projections ----
        with tc.tile_pool(name="dpsum", bufs=4, space="PSUM") as dpsum:
            for dt in range(DT):
                for slot, off, fr in rsegs:
                    pd = dpsum.tile([P, fr], F32, tag="pd")
                    for kd in range(JR):
                        nc.tensor.matmul(
                            pd[:], wdr[slot][:, dt, kd], h_r[:, kd, off : off + fr],
                            start=(kd == 0), stop=(kd == JR - 1),
                        )
                    osb = opool.tile([P, fr], F16, tag="osb")
                    nc.scalar.activation(
                        osb[:], pd[:], Ident, bias=b2[slot][:, dt : dt + 1]
                    )
                    nc.gpsimd.dma_start(
                        io["out_r"][ts(dt, P), off : off + fr], osb[:]
                    )
                for t in range(NT):
                    # chunk 0 is the deep quarter: 2 full + 3 quarter k-tiles
                    nk = SH_PER_CORE if t == 0 else SH_FULL
                    pd = dpsum.tile([P, 512], F32, tag="pd")
                    for kd in range(nk):
                        if kd < SH_FULL:
                            hsrc = h_sf[:, kd, ts(t, 512)]
                        else:
                            hsrc = h_sq[:, kd - SH_FULL, :]
                        nc.tensor.matmul(
                            pd[:], wds[:, dt, kd], hsrc,
                            start=(kd == 0), stop=(kd == nk - 1),
                        )
                    osb = opool.tile([P, 512], F16, tag="osb")
                    nc.scalar.activation(
                        osb[:], pd[:], Ident, bias=bs2[:, dt : dt + 1]
                    )
                    nc.gpsimd.dma_start(
                        io["out_s"][ts(dt, P), ts(t, 512)], osb[:]
                    )


def build_nc(reps=1, slots=SLOT_DEFAULT, mq=False, xtq="gpsimd"):
    nslots = len(slots)
    CR = sum(slots)
    nc = bacc.Bacc(None, target_bir_lowering=False, debug=False)
    io = {
        "xg": nc.declare_dram_parameter("xg", [P, KO, CR], BF16, isOutput=False),
        "xT": nc.declare_dram_parameter("xT", [P, KO, T], BF16, isOutput=False),
        "ws1t": nc.declare_dram_parameter(
            "ws1t", [P, SH_PER_CORE, KO, 2 * P], BF16, isOutput=False
        ),
        "wds": nc.declare_dram_parameter(
            "wds", [P, DT, SH_PER_CORE, P], BF16, isOutput=False
        ),
        "bs1t": nc.declare_dram_parameter(
            "bs1t", [P, 2 * SH_PER_CORE], F32, isOutput=False
        ),
        "bs2c": nc.declare_dram_parameter("bs2c", [P, DT], F32, isOutput=False),
        "out_r": nc.declare_dram_parameter("out_r", [D, CR], F16, isOutput=True),
        "out_s": nc.declare_dram_parameter("out_s", [D, T], F16, isOutput=True),
    }
    for s in range(nslots):
        io[f"w1t{s}"] = nc.declare_dram_parameter(
            f"w1t{s}", [P, JR, KO, 2 * P], BF16, isOutput=False
        )
        io[f"wdr{s}"] = nc.declare_dram_parameter(
            f"wdr{s}", [P, DT, JR, P], BF16, isOutput=False
        )
        io[f"b1t{s}"] = nc.declare_dram_parameter(
            f"b1t{s}", [P, 2 * JR], F32, isOutput=False
        )
        io[f"b2c{s}"] = nc.declare_dram_parameter(
            f"b2c{s}", [P, DT], F32, isOutput=False
        )
    with tile.TileContext(nc) as tc:
        for r in range(reps):
            _body(tc, io, slots, uid=r, mq=mq, xtq=xtq)
    nc.compile()
    return nc


def _part_tiles(vec, n_tiles):
    """[n_tiles*128] -> [128, n_tiles] (partition-tiled per-row constants)."""
    return np.ascontiguousarray(vec.reshape(n_tiles, P).T.astype(np.float32))


def _route(inputs):
    """Host gate: top-2 expert ids and combine weights per token."""
    x = np.asarray(inputs["x"], np.float32).reshape(T, D)
    gate_w = np.asarray(inputs["gate_w"], np.float32)
    gate_bias = np.asarray(inputs["gate_bias"], np.float32)
    logits = x @ gate_w.T
    scores = 1.0 / (1.0 + np.exp(-logits))
    sfc = scores + gate_bias[None, :]
    idx = np.argpartition(-sfc, TOPK - 1, axis=1)[:, :TOPK]   # [T, 2]
    w = np.take_along_axis(sfc, idx, axis=1)
    w = w / (w.sum(axis=1, keepdims=True) + 1e-20) * SCALE
    return idx, w


def _pack_feasible(counts, widths):
    """Per-expert bin-count choices packing token lists into 8 bins of each
    width (contiguous single-expert segments), or None if infeasible."""
    nslots = len(widths)
    opts = []
    for c in counts:
        o = set()
        if nslots == 2:
            s0, s1 = widths
            for n0 in range(0, 9):
                rem = c - n0 * s0
                n1 = -(-rem // s1) if rem > 0 else 0
                if n1 <= 8:
                    o.add((n0, n1))
        else:
            s0, s1, s2 = widths
            for n0 in range(0, 9):
                for n1 in range(0, 9):
                    rem = c - n0 * s0 - n1 * s1
                    n2 = -(-rem // s2) if rem > 0 else 0
                    if n2 <= 8:
                        o.add((n0, n1, n2))
        if not o:
            return None
        opts.append(sorted(o))
    # DP with parent pointers for traceback
    layers = [{tuple([0] * nslots): None}]
    for o in opts:
        ns = {}
        for st in layers[-1]:
            for ch in o:
                nxt = tuple(a + b for a, b in zip(st, ch))
                if all(v <= 8 for v in nxt) and nxt not in ns:
                    ns[nxt] = (st, ch)
        if not ns:
            return None
        layers.append(ns)
    st = next(iter(layers[-1]))
    choices = []
    for li in range(len(opts), 0, -1):
        prev, ch = layers[li][st]
        choices.append(ch)
        st = prev
    return list(reversed(choices))


def _slot_search(counts, nslots=2):
    """Minimal total slot width: scan totals ascending, return first feasible.

    Widths stay even so fp16/bf16 column offsets remain 4-byte aligned.
    """
    if nslots == 2:
        lo = -(-sum(counts) // N_CORES)
        lo += lo % 2
        for tot in range(lo, 2 * max(1024, lo) + 1, 2):
            s0_lo = tot - tot // 2
            s0_lo += s0_lo % 2
            for s0 in range(s0_lo, min(tot - 16, 1024) + 1, 2):
                wid = (s0, tot - s0)
                ch = _pack_feasible(counts, wid)
                if ch is not None:
                    # narrow slot first: the first matmul group then needs
                    # the least DMA, shrinking cold-start latency
                    return (wid[1], wid[0]), [(c[1], c[0]) for c in ch]
        raise AssertionError(f"no feasible slot config for counts {counts}")
    best = None
    for s0 in range(256, 1026, 8):
        for s1 in range(32, s0 + 1, 8):
            for wid in [(s0, s1, s2) for s2 in range(16, s1 + 1, 16)]:
                tot = sum(wid)
                if 8 * tot < 4096:
                    continue
                if best and tot >= sum(best[0]):
                    continue
                ch = _pack_feasible(counts, wid)
                if ch is not None:
                    best = (wid, ch)
    assert best is not None, f"no feasible slot config for counts {counts}"
    return best


def _pack_slots(counts, widths, choices):
    """Assign token segments to bins.

    Returns bins[slot_type][core] = (expert, start, stop) — a slice into that
    expert's own token list; (-1, 0, 0) marks an empty bin.
    """
    nslots = len(widths)
    bins = [[] for _ in range(nslots)]
    for e in range(N_CORES):
        pos = 0
        for s in range(nslots):
            for _ in range(choices[e][s]):
                seg = min(widths[s], max(0, counts[e] - pos))
                bins[s].append((e, pos, pos + seg))
                pos += seg
        assert pos >= counts[e]
    for s in range(nslots):
        assert len(bins[s]) <= N_CORES
        bins[s] += [(-1, 0, 0)] * (N_CORES - len(bins[s]))
    return bins


def prep_inputs(inputs, slots=None):
    """Full problem inputs -> (list of 8 per-core in_maps, dispatch, slots)."""
    x = np.asarray(inputs["x"], np.float32)
    W1 = np.asarray(inputs["W1"], np.float32)
    b1 = np.asarray(inputs["b1"], np.float32)
    W2 = np.asarray(inputs["W2"], np.float32)
    b2 = np.asarray(inputs["b2"], np.float32)
    Ws1 = np.asarray(inputs["Ws1"], np.float32)
    bs1 = np.asarray(inputs["bs1"], np.float32)
    Ws2 = np.asarray(inputs["Ws2"], np.float32)
    bs2 = np.asarray(inputs["bs2"], np.float32)

    xf = x.reshape(T, D)
    topk_idx, topk_w = _route(inputs)
    flat_e = topk_idx.ravel()
    flat_t = np.repeat(np.arange(T), TOPK)
    flat_w = topk_w.ravel()
    counts = np.bincount(flat_e, minlength=E)
    order = np.argsort(flat_e, kind="stable")
    bounds = np.concatenate([[0], np.cumsum(counts)])
    etok = [flat_t[order[bounds[e] : bounds[e + 1]]] for e in range(E)]
    ewgt = [flat_w[order[bounds[e] : bounds[e + 1]]] for e in range(E)]

    if slots is None:
        slots, choices = _slot_search(counts, nslots=2)
    else:
        choices = _pack_feasible(counts, tuple(slots))
        assert choices is not None, f"slots {slots} infeasible for {counts}"
    widths = tuple(slots)
    nslots = len(widths)
    CR = sum(widths)
    offs = np.concatenate([[0], np.cumsum(widths)])
    bins = _pack_slots(counts, widths, choices)

    # xT_prep[p, ko, t] = xf[t, ko*128+p]; per-roll variants for the shared
    # deep quarter (cores c and c+4 share a roll).
    xT16 = np.ascontiguousarray(
        xf.T.reshape(KO, P, T).transpose(1, 0, 2)
    ).astype(NP_BF16)
    xT_roll = {r: np.ascontiguousarray(np.roll(xT16, -r, axis=2))
               for r in sorted(set(ROLLS))}

    def routed_up(e):
        A = W1[e].reshape(2, JR, P, KO, P)  # (vg, j, m, ko, p)
        w1t = np.ascontiguousarray(
            A.transpose(4, 1, 3, 0, 2).reshape(P, JR, KO, 2 * P)
        ).astype(NP_BF16)
        b1t = np.ascontiguousarray(
            b1[e].reshape(2, JR, P).transpose(2, 1, 0).reshape(P, 2 * JR)
        ).astype(np.float32)
        return w1t, b1t

    def routed_down(e):
        wdr = np.ascontiguousarray(
            W2[e].T.reshape(JR, P, DT, P).transpose(1, 2, 0, 3)
        ).astype(NP_BF16)
        return wdr, _part_tiles(b2[e], DT)

    zero_up = (np.zeros((P, JR, KO, 2 * P), NP_BF16),
               np.zeros((P, 2 * JR), np.float32))
    zero_down = (np.zeros((P, DT, JR, P), NP_BF16),
                 np.zeros((P, DT), np.float32))
    up_cache = {e: routed_up(e) for e in range(E)}
    down_cache = {e: routed_down(e) for e in range(E)}
    up_cache[-1], down_cache[-1] = zero_up, zero_down

    in_maps, dispatch = [], []
    for c in range(N_CORES):
        segs = [bins[s][c] for s in range(nslots)]
        toks, wgts = [], []
        xg_full = np.zeros((CR, D), np.float32)
        for s, (e, a0, a1) in enumerate(segs):
            tk = etok[e][a0:a1] if e >= 0 else np.zeros(0, np.int64)
            wg = ewgt[e][a0:a1] if e >= 0 else np.zeros(0, np.float32)
            toks.append(tk)
            wgts.append(wg)
            xg_full[offs[s] : offs[s] + len(tk)] = xf[tk]
        dispatch.append((toks, wgts))
        xg16 = np.ascontiguousarray(
            xg_full.T.reshape(KO, P, CR).transpose(1, 0, 2)
        ).astype(NP_BF16)

        # shared expert: 2 full tiles + 3 deep-quarter tiles
        tiles = [2 * c, 2 * c + 1] + (
            [16, 17, 18] if c < 4 else [19, 20, 21]
        )
        A_sh = np.zeros((2, SH_PER_CORE, P, D), np.float32)
        bs1t_raw = np.zeros((2, SH_PER_CORE, P), np.float32)
        Wd_sh = np.zeros((SH_PER_CORE, P, D), np.float32)
        for jj, jglob in enumerate(tiles):
            rows = slice(jglob * P, (jglob + 1) * P)
            A_sh[0, jj] = Ws1[rows]
            A_sh[1, jj] = Ws1[I_SH + rows.start : I_SH + rows.stop]
            bs1t_raw[0, jj] = bs1[rows]
            bs1t_raw[1, jj] = bs1[I_SH + rows.start : I_SH + rows.stop]
            Wd_sh[jj] = Ws2[:, rows].T
        ws1t = np.ascontiguousarray(
            A_sh.reshape(2, SH_PER_CORE, P, KO, P)
            .transpose(4, 1, 3, 0, 2)
            .reshape(P, SH_PER_CORE, KO, 2 * P)
        ).astype(NP_BF16)
        bs1t = np.ascontiguousarray(
            bs1t_raw.transpose(2, 1, 0).reshape(P, 2 * SH_PER_CORE)
        ).astype(np.float32)
        wds = np.ascontiguousarray(
            Wd_sh.reshape(SH_PER_CORE * P, D)
            .reshape(SH_PER_CORE, P, DT, P)
            .transpose(1, 2, 0, 3)
        ).astype(NP_BF16)

        bs2_c = bs2 if c == 0 else np.zeros_like(bs2)

        m = {
            "xg": xg16,
            "xT": xT_roll[ROLLS[c]],
            "ws1t": ws1t,
            "wds": wds,
            "bs1t": bs1t,
            "bs2c": _part_tiles(bs2_c, DT),
        }
        for s, (e, _, _) in enumerate(segs):
            m[f"w1t{s}"], m[f"b1t{s}"] = up_cache[e]
            m[f"wdr{s}"], m[f"b2c{s}"] = down_cache[e]
        in_maps.append(m)
    return in_maps, dispatch, widths


_NC_CACHE = {}


MQ_DEFAULT = False


def get_nc(slots=SLOT_DEFAULT, mq=None, xtq="gpsimd"):
    if mq is None:
        mq = MQ_DEFAULT
    key = ("nc", tuple(slots), mq, xtq)
    if key not in _NC_CACHE:
        _NC_CACHE[key] = build_nc(slots=slots, mq=mq, xtq=xtq)
    return _NC_CACHE[key]


def combine_outputs(results, dispatch, slots):
    """Per-core result dicts -> full [B, S, D] float32 output."""
    offs = np.concatenate([[0], np.cumsum(slots)])
    acc = np.zeros((D, T), np.float32)
    for c, r in enumerate(results):
        acc += np.roll(np.asarray(r["out_s"], np.float32), ROLLS[c], axis=1)
    for r, (toks, wgts) in zip(results, dispatch):
        out_r = np.asarray(r["out_r"], np.float32)
        for s, (tk, wg) in enumerate(zip(toks, wgts)):
            n = len(tk)
            if n:
                acc[:, tk] += out_r[:, offs[s] : offs[s] + n] * wg[None, :]
    return np.ascontiguousarray(acc.T.reshape(B, S, D))


def kernel(**inputs):
    in_maps, dispatch, slots = prep_inputs(inputs)
    nc = get_nc(slots)
    res = run_bass_kernel_spmd(nc, in_maps, core_ids=list(range(N_CORES)))
    return combine_outputs(res.results, dispatch, slots)


if __name__ == "__main__":
    # quick self-drive (requires reference.py next to this file)
    import reference

    inputs = {k: np.asarray(v) for k, v in reference.setup_inputs().items()}
    out = kernel(**inputs)
    exp = np.asarray(reference.reference(**inputs))
    err = np.abs(out - exp).max()
    rel = np.abs(out - exp).max() / np.abs(exp).max()
    print("absmax err:", err, "rel:", rel)
